# revision 61
# baseline (speedup 1.0000x reference)
# Trainium2 Bass kernel for nn_Attn_7413113553589 (sparse_attention).
#
# Reference computation:
#   wh   = einsum('lbi,hi->lbh', x, W_lin) + b_lin        [L,B,H]
#   vwh  = einsum('fh,lbh->lbf', v, wh)                   [L,B,FP]
#   wh2  = einsum('lbi,ei->lbe', x, W_feat) + b_feat      [L,B,E]
#   fwh  = einsum('ne,lbe->lbn', FE, wh2)                 [L,B,NF]
#   attn = softmax(concat([fwh, vwh], -1))                [L,B,NF+FP]
#   out  = attn.transpose(1,2,0)                          [B,NF+FP,L]
#
# Algebraic fold: the two linear chains collapse into a single combined
# matrix applied to every token:
#   A^T[i,n] = concat_n( (FE @ W_feat)^T , (v @ W_lin)^T )   [2H, NF+FP]
#   c[n]     = concat( FE @ b_feat , v @ b_lin )             [NF+FP]
#   energy[t,n] = x[t,:] @ A^T + c[n];  attn = softmax_n(energy)
# The fold itself is tiny (~2.5% of total FLOPs) and runs on the host by
# default (HOST_FOLD) in fp32; a full on-device fold path is kept as well.
# The bias c is applied as a ninth K=1 accumulation step (ones_row.T @ c).
#
# Sharding: data-parallel over batch. 8 cores x 4 batches each; weights
# replicated. Each core computes [4, 1024, 512] of the output.
#
# Per-core pipeline (fp16 matmul inputs, fp32 PSUM accumulation):
#   phase 0: load A^T (8 k-tiles) + the bias row.
#   phase 1: per 128-token tile (one batch b, 128 l values):
#     cast-DMA x tile [128t, 1024i] fp32->fp16 (SWDGE)
#     x^T k-tiles via PE transpose (identity rhs) + one DVE copy to SBUF
#     energy[t, n-half] = sum_k x^T_k.T @ A^T_k + 1.T @ c  (PE, fp32 PSUM)
#     -max_n (DVE reduce, negated)
#     exp(e - max) -> fp16 with fused row-sum (ACT activation + accum_out)
#     attn^T tiles = exp_tile.T @ diag(1/sum)  (PE: transpose+scale in one mm)
#     ACT copies PSUM -> SBUF staging; one 2 MB DMA per batch (SP ring).
# Engine-copy assignment matters: the ACT queue is strict FIFO and exp
# releases the energy PSUM slot, so the bulky x^T copy lives on DVE and the
# output copy on ACT (A/B-tested in the cost model).

import numpy as np

L, B, H2 = 512, 32, 1024
NF, FP, E, H = 768, 256, 512, 512
N = NF + FP  # 1024
NCORES = 8
BPC = B // NCORES  # batches per core = 4

_cache = {}

# If True, the weight fold (A^T = [FE@W_feat | v@W_lin]^T and the bias row c)
# is precomputed on the host in fp32 and shipped as fp16 inputs; the device
# then runs only the per-token pipeline. If False, the fold runs on-device.
HOST_FOLD = True


def _build(repeat=1, host_fold=HOST_FOLD):
    import concourse.bass as bass
    import concourse.tile as tile
    from concourse import bacc, mybir

    f16 = mybir.dt.float16
    f32 = mybir.dt.float32

    nc = bacc.Bacc(
        "TRN2", target_bir_lowering=False, debug=False, enable_asserts=False,
        num_devices=NCORES,
    )

    x_d = nc.dram_tensor("x", [L, BPC, H2], f32, kind="ExternalInput").ap()
    if host_fold:
        at_d = nc.dram_tensor("atw", [8, 128, N], f16, kind="ExternalInput").ap()
        wr_d = nc.dram_tensor("wrow", [1, N], f16, kind="ExternalInput").ap()
        ws = (at_d, wr_d)
    else:
        fe_d = nc.dram_tensor("fe", [NF, E], f32, kind="ExternalInput").ap()
        wl_d = nc.dram_tensor("wlin", [H, H2], f32, kind="ExternalInput").ap()
        bl_d = nc.dram_tensor("blin", [H], f32, kind="ExternalInput").ap()
        wf_d = nc.dram_tensor("wfeat", [E, H2], f32, kind="ExternalInput").ap()
        bf_d = nc.dram_tensor("bfeat", [E], f32, kind="ExternalInput").ap()
        v_d = nc.dram_tensor("v", [FP, H], f32, kind="ExternalInput").ap()
        ws = (fe_d, wl_d, bl_d, wf_d, bf_d, v_d)
    out_d = nc.dram_tensor("out", [BPC, N, L], f32, kind="ExternalOutput").ap()

    ident_h = nc.inline_tensor(np.eye(128, dtype=np.float16), name="ident128")

    with tile.TileContext(nc) as tc:
        _body(tc, x_d, ws, out_d, ident_h, f16, f32, mybir, bass, repeat,
              host_fold)

    nc.compile()
    return nc


def _body(tc, x_d, ws, out_d, ident_h, f16, f32, mybir, bass, repeat,
          host_fold):
    from contextlib import ExitStack

    nc = tc.nc
    Exp = mybir.ActivationFunctionType.Exp

    with ExitStack() as ctx:
        singles = ctx.enter_context(tc.tile_pool(name="singles", bufs=1))
        wpool = ctx.enter_context(tc.tile_pool(name="wpool", bufs=1))
        xpool = ctx.enter_context(tc.tile_pool(name="xpool", bufs=4))
        xtpool = ctx.enter_context(tc.tile_pool(name="xtpool", bufs=4))
        epool = ctx.enter_context(tc.tile_pool(name="epool", bufs=3))
        stats = ctx.enter_context(tc.tile_pool(name="stats", bufs=8))
        opool = ctx.enter_context(tc.tile_pool(name="opool", bufs=2))
        # PSUM pools: psB 2x1 + psA 2x1 + psC 2x1 + psO 1x2 = 8 banks.
        # The two energy halves live in separate pools so each releases for
        # tile i+2 as soon as its own exp half has read it.
        psB = ctx.enter_context(tc.tile_pool(name="psB", bufs=2, space="PSUM"))
        psA = ctx.enter_context(tc.tile_pool(name="psA", bufs=2, space="PSUM"))
        psC = ctx.enter_context(tc.tile_pool(name="psC", bufs=2, space="PSUM"))
        psO = ctx.enter_context(tc.tile_pool(name="psO", bufs=1, space="PSUM"))

        # ---- constants ----
        ident = singles.tile([128, 128], f16)
        nc.sync.dma_start(out=ident, in_=ident_h.ap())
        ones_row = singles.tile([1, 128], f16)
        nc.vector.memset(ones_row, 1.0)

        # ---- phase 0: load (+ maybe fold) weights ----
        x_r = x_d.rearrange("(lb p) b i -> b p lb i", p=128)  # [4, 128, 4, 1024]
        x16_tiles = {}

        def load_x(b, split=False):
            if split:
                # per-l-block chunks: lets tile 0's transposes start ~4x sooner
                parts = []
                for lb in range(4):
                    t = xpool.tile([128, H2], f16, tag="x16s")
                    nc.gpsimd.dma_start(out=t, in_=x_r[b][:, lb, :])
                    parts.append(t)
                x16_tiles[b] = parts
            else:
                t = xpool.tile([128, 4, H2], f16, tag="x16")
                x16_tiles[b] = t
                nc.gpsimd.dma_start(out=t, in_=x_r[b])

        if host_fold:
            at_d, wr_d = ws
            # bias row first: it is tiny and feeds every energy tile.
            bias_row = wpool.tile([1, N], f16)
            nc.sync.dma_start(out=bias_row, in_=wr_d)
            load_x(0, split=True)
            at = []
            for it in range(8):
                atk = wpool.tile([128, N], f16, tag=f"at{it}")
                nc.sync.dma_start(out=atk, in_=at_d[it])
                at.append(atk)
            _main_loop(tc, out_d, f16, f32, mybir, repeat, load_x,
                       x16_tiles, at, bias_row, ones_row, ident,
                       xtpool, epool, stats, opool, psA, psB, psC, psO)
            return

        fe_d, wl_d, bl_d, wf_d, bf_d, v_d = ws
        # fe first: it heads the critical chain (fe -> FE^T -> A^T -> matmul).
        fe16 = wpool.tile([128, 6, E], f16)  # [n1%128, n1//128, e]
        nc.gpsimd.dma_start(out=fe16, in_=fe_d.rearrange("(t p) e -> p t e", p=128))
        # x batch 0 right behind fe so tile 0's transposes can start early
        load_x(0)
        wf16 = wpool.tile([128, 4, H2], f16)  # [e%128, e//128, i]
        nc.gpsimd.dma_start(out=wf16, in_=wf_d.rearrange("(t p) i -> p t i", p=128))
        wl16 = wpool.tile([128, 4, H2], f16)  # [h%128, h//128, i]
        nc.gpsimd.dma_start(out=wl16, in_=wl_d.rearrange("(t p) i -> p t i", p=128))
        v16 = wpool.tile([128, 2, H], f16)  # [n2%128, n2//128, h]
        nc.gpsimd.dma_start(out=v16, in_=v_d.rearrange("(t p) h -> p t h", p=128))
        bf16 = wpool.tile([128, 4], f16)  # [e%128, e//128]
        nc.gpsimd.dma_start(out=bf16, in_=bf_d.rearrange("(t p) -> p t", p=128))
        bl16 = wpool.tile([128, 4], f16)
        nc.gpsimd.dma_start(out=bl16, in_=bl_d.rearrange("(t p) -> p t", p=128))

        # FE^T: [e, n1] as 4 e-tiles x 768
        fet = wpool.tile([128, 4, NF], f16)
        for et in range(4):
            ps = psB.tile([128, NF], f16, tag="pstr")
            for nt in range(6):
                nc.tensor.transpose(
                    ps[:, nt * 128:(nt + 1) * 128],
                    fe16[:, nt, et * 128:(et + 1) * 128], ident)
            nc.scalar.copy(fet[:, et, :], ps)
        # v^T: [h, n2] as 4 h-tiles x 256
        vt = wpool.tile([128, 4, FP], f16)
        for ht in range(4):
            ps = psB.tile([128, FP], f16, tag="pstr")
            for nt in range(2):
                nc.tensor.transpose(
                    ps[:, nt * 128:(nt + 1) * 128],
                    v16[:, nt, ht * 128:(ht + 1) * 128], ident)
            nc.scalar.copy(vt[:, ht, :], ps)

        # A^T [i, n] built per i-tile; stored fp16 as 8 separate tiles so the
        # main matmul's k-loop can start as soon as its own k-tile is ready.
        at = []
        for it in range(8):
            isl = slice(it * 128, (it + 1) * 128)
            atk = wpool.tile([128, N], f16, tag=f"at{it}")
            ps0 = psA.tile([128, 512], f32, tag="pe0")
            for et in range(4):
                nc.tensor.matmul(ps0, wf16[:, et, isl], fet[:, et, 0:512],
                                 start=(et == 0), stop=(et == 3))
            nc.scalar.copy(atk[:, 0:512], ps0)
            ps1 = psC.tile([128, 512], f32, tag="pe1")
            for et in range(4):
                nc.tensor.matmul(ps1[:, 0:256], wf16[:, et, isl],
                                 fet[:, et, 512:768],
                                 start=(et == 0), stop=(et == 3))
            for ht in range(4):
                nc.tensor.matmul(ps1[:, 256:512], wl16[:, ht, isl],
                                 vt[:, ht, :],
                                 start=(ht == 0), stop=(ht == 3))
            nc.scalar.copy(atk[:, 512:1024], ps1)
            at.append(atk)

        # bias row c[n], fp16 (used as the K=1 ninth accumulation step)
        bias_row = wpool.tile([1, N], f16)
        pc = psB.tile([1, 512], f32, tag="pstr")
        for et in range(4):
            nc.tensor.matmul(pc, bf16[:, et:et + 1], fet[:, et, 0:512],
                             start=(et == 0), stop=(et == 3))
        nc.scalar.copy(bias_row[:, 0:512], pc)
        pc = psB.tile([1, 512], f32, tag="pstr")
        for et in range(4):
            nc.tensor.matmul(pc[:, 0:256], bf16[:, et:et + 1],
                             fet[:, et, 512:768],
                             start=(et == 0), stop=(et == 3))
        for ht in range(4):
            nc.tensor.matmul(pc[:, 256:512], bl16[:, ht:ht + 1], vt[:, ht, :],
                             start=(ht == 0), stop=(ht == 3))
        nc.scalar.copy(bias_row[:, 512:1024], pc)

        _main_loop(tc, out_d, f16, f32, mybir, repeat, load_x,
                   x16_tiles, at, bias_row, ones_row, ident,
                   xtpool, epool, stats, opool, psA, psB, psC, psO)


def _main_loop(tc, out_d, f16, f32, mybir, repeat, load_x, x16_tiles,
               at, bias_row, ones_row, ident,
               xtpool, epool, stats, opool, psA, psB, psC, psO):
    nc = tc.nc
    Exp = mybir.ActivationFunctionType.Exp
    out_r = out_d.rearrange("b (j p) (lb l) -> b p j lb l", p=128, l=128)

    for _rep in range(repeat):
        if _rep > 0:
            load_x(0)
        for b in range(BPC):
            x16 = x16_tiles[b]

            out_sb = opool.tile([128, 8, 4, 128], f32)  # [n%128, n//128, lb, l]
            for lb in range(4):
                if lb == 2 and b + 1 < BPC:
                    load_x(b + 1)  # prefetch next batch mid-way through this one
                xsrc = x16[lb] if isinstance(x16, list) else x16[:, lb, :]
                pst = psB.tile([128, H2], f16, tag="pstr")
                for k in range(8):
                    ksl = slice(k * 128, (k + 1) * 128)
                    nc.tensor.transpose(pst[:, ksl], xsrc[:, ksl], ident)
                xt = xtpool.tile([128, H2], f16)
                nc.vector.tensor_copy(xt, pst)

                pehalves = []
                nmh = []
                for half, pool in ((0, psA), (1, psC)):
                    nsl = slice(half * 512, (half + 1) * 512)
                    peh = pool.tile([128, 512], f32, tag=f"pe{half}")
                    for k in range(8):
                        nc.tensor.matmul(peh, xt[:, k * 128:(k + 1) * 128],
                                         at[k][:, nsl],
                                         start=(k == 0), stop=False)
                    # ninth step: broadcast the bias row via a K=1 matmul
                    nc.tensor.matmul(peh, ones_row, bias_row[:, nsl],
                                     start=False, stop=True)
                    nm = stats.tile([128, 1], f32, tag="nmh")
                    nc.vector.tensor_reduce(
                        nm, peh, axis=mybir.AxisListType.X,
                        op=mybir.AluOpType.max, negate=True)
                    pehalves.append(peh)
                    nmh.append(nm)
                negmax = stats.tile([128, 1], f32)
                nc.vector.tensor_tensor(negmax, nmh[0], nmh[1],
                                        op=mybir.AluOpType.min)

                expv = epool.tile([128, N], f16)
                shalf = []
                for half in range(2):
                    nsl = slice(half * 512, (half + 1) * 512)
                    sh = stats.tile([128, 1], f32, tag="sh")
                    nc.scalar.activation(expv[:, nsl], pehalves[half], Exp,
                                         bias=negmax, scale=1.0, accum_out=sh)
                    shalf.append(sh)
                sums = stats.tile([128, 1], f32)
                nc.vector.tensor_add(sums, shalf[0], shalf[1])
                invs = stats.tile([128, 1], f32)
                nc.vector.reciprocal(invs, sums)
                diag = stats.tile([128, 128], f16, tag="diag")
                nc.vector.tensor_scalar_mul(diag, ident, invs)

                po = psO.tile([128, N], f32)
                for j in range(8):
                    jsl = slice(j * 128, (j + 1) * 128)
                    nc.tensor.matmul(po[:, jsl], expv[:, jsl], diag,
                                     start=True, stop=True)
                nc.scalar.copy(
                    out_sb[:, :, lb, :],
                    po.rearrange("p (j l) -> p j l", j=8))

                if b == BPC - 1:
                    # split the final batch's store so the tail drains early
                    nc.sync.dma_start(out=out_r[b, :, :, lb, :],
                                      in_=out_sb[:, :, lb, :])
            if b != BPC - 1:
                nc.sync.dma_start(out=out_r[b], in_=out_sb)


def _fold(inputs):
    FE = np.asarray(inputs["feature_embeddings"], np.float32)
    W_lin = np.asarray(inputs["W_lin"], np.float32)
    b_lin = np.asarray(inputs["b_lin"], np.float32)
    W_feat = np.asarray(inputs["W_feat"], np.float32)
    b_feat = np.asarray(inputs["b_feat"], np.float32)
    v = np.asarray(inputs["v"], np.float32)
    a1t = W_feat.T @ FE.T              # [2H, NF]
    a2t = W_lin.T @ v.T                # [2H, FP]
    at = np.concatenate([a1t, a2t], axis=1)          # [2H, N]
    c = np.concatenate([FE @ b_feat, v @ b_lin])     # [N]
    atw = np.ascontiguousarray(
        at.reshape(8, 128, N).astype(np.float16))
    wrow = c.astype(np.float16).reshape(1, N)  # bias row
    return atw, wrow


def _shards(inputs, host_fold=HOST_FOLD):
    x = np.ascontiguousarray(inputs["encoder_outputs"], dtype=np.float32)
    if host_fold:
        atw, wrow = _fold(inputs)
        rep = {"atw": atw, "wrow": wrow}
    else:
        rep = {
            "fe": np.ascontiguousarray(inputs["feature_embeddings"], np.float32),
            "wlin": np.ascontiguousarray(inputs["W_lin"], np.float32),
            "blin": np.ascontiguousarray(inputs["b_lin"], np.float32),
            "wfeat": np.ascontiguousarray(inputs["W_feat"], np.float32),
            "bfeat": np.ascontiguousarray(inputs["b_feat"], np.float32),
            "v": np.ascontiguousarray(inputs["v"], np.float32),
        }
    in_maps = []
    for c in range(NCORES):
        m = dict(rep)
        m["x"] = np.ascontiguousarray(x[:, c * BPC:(c + 1) * BPC, :])
        in_maps.append(m)
    return in_maps


def kernel(**inputs):
    from concourse.bass_utils import run_bass_kernel_spmd

    if "nc" not in _cache:
        _cache["nc"] = _build()
    nc = _cache["nc"]
    res = run_bass_kernel_spmd(nc, _shards(inputs), core_ids=list(range(NCORES)))
    out = np.concatenate([r["out"] for r in res.results], axis=0)
    return out.astype(np.float32)


# revision 63
# speedup vs baseline: 1.0291x; 1.0291x over previous
# Trainium2 Bass kernel for nn_Attn_7413113553589 (sparse_attention).
#
# Reference computation:
#   wh   = einsum('lbi,hi->lbh', x, W_lin) + b_lin        [L,B,H]
#   vwh  = einsum('fh,lbh->lbf', v, wh)                   [L,B,FP]
#   wh2  = einsum('lbi,ei->lbe', x, W_feat) + b_feat      [L,B,E]
#   fwh  = einsum('ne,lbe->lbn', FE, wh2)                 [L,B,NF]
#   attn = softmax(concat([fwh, vwh], -1))                [L,B,NF+FP]
#   out  = attn.transpose(1,2,0)                          [B,NF+FP,L]
#
# Algebraic fold: the two linear chains collapse into a single combined
# matrix applied to every token:
#   A^T[i,n] = concat_n( (FE @ W_feat)^T , (v @ W_lin)^T )   [2H, NF+FP]
#   c[n]     = concat( FE @ b_feat , v @ b_lin )             [NF+FP]
#   energy[t,n] = x[t,:] @ A^T + c[n];  attn = softmax_n(energy)
# The fold itself is tiny (~2.5% of total FLOPs) and runs on the host by
# default (HOST_FOLD) in fp32; a full on-device fold path is kept as well.
# The bias c is applied as a ninth K=1 accumulation step (ones_row.T @ c).
#
# Sharding: data-parallel over batch. 8 cores x 4 batches each; weights
# replicated. Each core computes [4, 1024, 512] of the output.
#
# Per-core pipeline (fp16 matmul inputs, fp32 PSUM accumulation):
#   phase 0: load A^T (8 k-tiles) + the bias row.
#   phase 1: per 128-token tile (one batch b, 128 l values):
#     cast-DMA x tile [128t, 1024i] fp32->fp16 (SWDGE)
#     x^T k-tiles via PE transpose (identity rhs) + one DVE copy to SBUF
#     energy[t, n-half] = sum_k x^T_k.T @ A^T_k + 1.T @ c  (PE, fp32 PSUM)
#       - each 512-wide half accumulates in its OWN single-bank PSUM pool so
#         it can be released for tile i+2 as soon as its exp half reads it
#     -max_n per half (DVE reduce, negated) + min-combine
#     exp(e - max) -> fp16 per half with fused row-sums (ACT accum_out) + add
#     attn^T tiles = exp_tile.T @ diag(1/sum)  (PE: transpose+scale in one mm)
#     ACT copies PSUM -> SBUF staging; one 2 MB DMA per batch (SP ring).
# Engine-copy assignment matters: the ACT queue is strict FIFO and exp
# releases the energy PSUM slots, so the bulky x^T copy lives on DVE and the
# output copy on ACT (A/B-tested in the cost model; the other three
# assignments measure 4-6% slower).

import numpy as np

L, B, H2 = 512, 32, 1024
NF, FP, E, H = 768, 256, 512, 512
N = NF + FP  # 1024
NCORES = 8
BPC = B // NCORES  # batches per core = 4

_cache = {}

# If True, the weight fold (A^T = [FE@W_feat | v@W_lin]^T and the bias row c)
# is precomputed on the host in fp32 and shipped as fp16 inputs; the device
# then runs only the per-token pipeline. If False, the fold runs on-device.
HOST_FOLD = True


def _build(repeat=1, host_fold=HOST_FOLD):
    import concourse.bass as bass
    import concourse.tile as tile
    from concourse import bacc, mybir

    f16 = mybir.dt.float16
    f32 = mybir.dt.float32

    nc = bacc.Bacc(
        "TRN2", target_bir_lowering=False, debug=False, enable_asserts=False,
        num_devices=NCORES,
    )

    x_d = nc.dram_tensor("x", [L, BPC, H2], f32, kind="ExternalInput").ap()
    if host_fold:
        at_d = nc.dram_tensor("atw", [8, 128, N], f16, kind="ExternalInput").ap()
        wr_d = nc.dram_tensor("wrow", [1, N], f16, kind="ExternalInput").ap()
        ws = (at_d, wr_d)
    else:
        fe_d = nc.dram_tensor("fe", [NF, E], f32, kind="ExternalInput").ap()
        wl_d = nc.dram_tensor("wlin", [H, H2], f32, kind="ExternalInput").ap()
        bl_d = nc.dram_tensor("blin", [H], f32, kind="ExternalInput").ap()
        wf_d = nc.dram_tensor("wfeat", [E, H2], f32, kind="ExternalInput").ap()
        bf_d = nc.dram_tensor("bfeat", [E], f32, kind="ExternalInput").ap()
        v_d = nc.dram_tensor("v", [FP, H], f32, kind="ExternalInput").ap()
        ws = (fe_d, wl_d, bl_d, wf_d, bf_d, v_d)
    out_d = nc.dram_tensor("out", [BPC, N, L], f32, kind="ExternalOutput").ap()

    ident_h = nc.inline_tensor(np.eye(128, dtype=np.float16), name="ident128")

    with tile.TileContext(nc) as tc:
        _body(tc, x_d, ws, out_d, ident_h, f16, f32, mybir, bass, repeat,
              host_fold)

    nc.compile()
    return nc


def _body(tc, x_d, ws, out_d, ident_h, f16, f32, mybir, bass, repeat,
          host_fold):
    from contextlib import ExitStack

    nc = tc.nc
    Exp = mybir.ActivationFunctionType.Exp

    with ExitStack() as ctx:
        singles = ctx.enter_context(tc.tile_pool(name="singles", bufs=1))
        wpool = ctx.enter_context(tc.tile_pool(name="wpool", bufs=1))
        xpool = ctx.enter_context(tc.tile_pool(name="xpool", bufs=4))
        xtpool = ctx.enter_context(tc.tile_pool(name="xtpool", bufs=4))
        epool = ctx.enter_context(tc.tile_pool(name="epool", bufs=3))
        stats = ctx.enter_context(tc.tile_pool(name="stats", bufs=8))
        opool = ctx.enter_context(tc.tile_pool(name="opool", bufs=2))
        # PSUM pools: psB 2x1 + psA 2x1 + psC 2x1 + psO 1x2 = 8 banks.
        # The two energy halves live in separate pools so each releases for
        # tile i+2 as soon as its own exp half has read it.
        psB = ctx.enter_context(tc.tile_pool(name="psB", bufs=2, space="PSUM"))
        psA = ctx.enter_context(tc.tile_pool(name="psA", bufs=2, space="PSUM"))
        psC = ctx.enter_context(tc.tile_pool(name="psC", bufs=2, space="PSUM"))
        psO = ctx.enter_context(tc.tile_pool(name="psO", bufs=1, space="PSUM"))

        # ---- constants ----
        ident = singles.tile([128, 128], f16)
        nc.sync.dma_start(out=ident, in_=ident_h.ap())
        ones_row = singles.tile([1, 128], f16)
        nc.vector.memset(ones_row, 1.0)

        # ---- phase 0: load (+ maybe fold) weights ----
        x_r = x_d.rearrange("(lb p) b i -> b p lb i", p=128)  # [4, 128, 4, 1024]
        x16_tiles = {}

        def load_x(b, split=False):
            if split:
                # per-l-block chunks: lets tile 0's transposes start ~4x sooner
                parts = []
                for lb in range(4):
                    t = xpool.tile([128, H2], f16, tag="x16s")
                    nc.gpsimd.dma_start(out=t, in_=x_r[b][:, lb, :])
                    parts.append(t)
                x16_tiles[b] = parts
            else:
                t = xpool.tile([128, 4, H2], f16, tag="x16")
                x16_tiles[b] = t
                nc.gpsimd.dma_start(out=t, in_=x_r[b])

        if host_fold:
            at_d, wr_d = ws
            # bias row first: it is tiny and feeds every energy tile.
            bias_row = wpool.tile([1, N], f16)
            nc.sync.dma_start(out=bias_row, in_=wr_d)
            load_x(0, split=True)
            at = []
            for it in range(8):
                atk = wpool.tile([128, N], f16, tag=f"at{it}")
                nc.sync.dma_start(out=atk, in_=at_d[it])
                at.append(atk)
            _main_loop(tc, out_d, f16, f32, mybir, repeat, load_x,
                       x16_tiles, at, bias_row, ones_row, ident,
                       xtpool, epool, stats, opool, psA, psB, psC, psO)
            return

        fe_d, wl_d, bl_d, wf_d, bf_d, v_d = ws
        # fe first: it heads the critical chain (fe -> FE^T -> A^T -> matmul).
        fe16 = wpool.tile([128, 6, E], f16)  # [n1%128, n1//128, e]
        nc.gpsimd.dma_start(out=fe16, in_=fe_d.rearrange("(t p) e -> p t e", p=128))
        # x batch 0 right behind fe so tile 0's transposes can start early
        load_x(0)
        wf16 = wpool.tile([128, 4, H2], f16)  # [e%128, e//128, i]
        nc.gpsimd.dma_start(out=wf16, in_=wf_d.rearrange("(t p) i -> p t i", p=128))
        wl16 = wpool.tile([128, 4, H2], f16)  # [h%128, h//128, i]
        nc.gpsimd.dma_start(out=wl16, in_=wl_d.rearrange("(t p) i -> p t i", p=128))
        v16 = wpool.tile([128, 2, H], f16)  # [n2%128, n2//128, h]
        nc.gpsimd.dma_start(out=v16, in_=v_d.rearrange("(t p) h -> p t h", p=128))
        bf16 = wpool.tile([128, 4], f16)  # [e%128, e//128]
        nc.gpsimd.dma_start(out=bf16, in_=bf_d.rearrange("(t p) -> p t", p=128))
        bl16 = wpool.tile([128, 4], f16)
        nc.gpsimd.dma_start(out=bl16, in_=bl_d.rearrange("(t p) -> p t", p=128))

        # FE^T: [e, n1] as 4 e-tiles x 768
        fet = wpool.tile([128, 4, NF], f16)
        for et in range(4):
            ps = psB.tile([128, NF], f16, tag="pstr")
            for nt in range(6):
                nc.tensor.transpose(
                    ps[:, nt * 128:(nt + 1) * 128],
                    fe16[:, nt, et * 128:(et + 1) * 128], ident)
            nc.scalar.copy(fet[:, et, :], ps)
        # v^T: [h, n2] as 4 h-tiles x 256
        vt = wpool.tile([128, 4, FP], f16)
        for ht in range(4):
            ps = psB.tile([128, FP], f16, tag="pstr")
            for nt in range(2):
                nc.tensor.transpose(
                    ps[:, nt * 128:(nt + 1) * 128],
                    v16[:, nt, ht * 128:(ht + 1) * 128], ident)
            nc.scalar.copy(vt[:, ht, :], ps)

        # A^T [i, n] built per i-tile; stored fp16 as 8 separate tiles so the
        # main matmul's k-loop can start as soon as its own k-tile is ready.
        at = []
        for it in range(8):
            isl = slice(it * 128, (it + 1) * 128)
            atk = wpool.tile([128, N], f16, tag=f"at{it}")
            ps0 = psA.tile([128, 512], f32, tag="pe0")
            for et in range(4):
                nc.tensor.matmul(ps0, wf16[:, et, isl], fet[:, et, 0:512],
                                 start=(et == 0), stop=(et == 3))
            nc.scalar.copy(atk[:, 0:512], ps0)
            ps1 = psC.tile([128, 512], f32, tag="pe1")
            for et in range(4):
                nc.tensor.matmul(ps1[:, 0:256], wf16[:, et, isl],
                                 fet[:, et, 512:768],
                                 start=(et == 0), stop=(et == 3))
            for ht in range(4):
                nc.tensor.matmul(ps1[:, 256:512], wl16[:, ht, isl],
                                 vt[:, ht, :],
                                 start=(ht == 0), stop=(ht == 3))
            nc.scalar.copy(atk[:, 512:1024], ps1)
            at.append(atk)

        # bias row c[n], fp16 (used as the K=1 ninth accumulation step)
        bias_row = wpool.tile([1, N], f16)
        pc = psB.tile([1, 512], f32, tag="pstr")
        for et in range(4):
            nc.tensor.matmul(pc, bf16[:, et:et + 1], fet[:, et, 0:512],
                             start=(et == 0), stop=(et == 3))
        nc.scalar.copy(bias_row[:, 0:512], pc)
        pc = psB.tile([1, 512], f32, tag="pstr")
        for et in range(4):
            nc.tensor.matmul(pc[:, 0:256], bf16[:, et:et + 1],
                             fet[:, et, 512:768],
                             start=(et == 0), stop=(et == 3))
        for ht in range(4):
            nc.tensor.matmul(pc[:, 256:512], bl16[:, ht:ht + 1], vt[:, ht, :],
                             start=(ht == 0), stop=(ht == 3))
        nc.scalar.copy(bias_row[:, 512:1024], pc)

        _main_loop(tc, out_d, f16, f32, mybir, repeat, load_x,
                   x16_tiles, at, bias_row, ones_row, ident,
                   xtpool, epool, stats, opool, psA, psB, psC, psO)


def _main_loop(tc, out_d, f16, f32, mybir, repeat, load_x, x16_tiles,
               at, bias_row, ones_row, ident,
               xtpool, epool, stats, opool, psA, psB, psC, psO):
    nc = tc.nc
    Exp = mybir.ActivationFunctionType.Exp
    out_r = out_d.rearrange("b (j p) (lb l) -> b p j lb l", p=128, l=128)

    for _rep in range(repeat):
        if _rep > 0:
            load_x(0)
        for b in range(BPC):
            x16 = x16_tiles[b]

            out_sb = opool.tile([128, 8, 4, 128], f32)  # [n%128, n//128, lb, l]
            for lb in range(4):
                if lb == 2 and b + 1 < BPC:
                    load_x(b + 1)  # prefetch next batch mid-way through this one
                xsrc = x16[lb] if isinstance(x16, list) else x16[:, lb, :]
                pst = psB.tile([128, H2], f16, tag="pstr")
                for k in range(8):
                    ksl = slice(k * 128, (k + 1) * 128)
                    nc.tensor.transpose(pst[:, ksl], xsrc[:, ksl], ident)
                xt = xtpool.tile([128, H2], f16)
                nc.vector.tensor_copy(xt, pst)

                pehalves = []
                nmh = []
                for half, pool in ((0, psA), (1, psC)):
                    nsl = slice(half * 512, (half + 1) * 512)
                    peh = pool.tile([128, 512], f32, tag=f"pe{half}")
                    # bias first (K=1 broadcast): keeps the group-closing
                    # matmul on the k-loop so the reduce isn't gated on it,
                    # and lets the scheduler run it early into PE bubbles
                    nc.tensor.matmul(peh, ones_row, bias_row[:, nsl],
                                     start=True, stop=False)
                    for k in range(8):
                        nc.tensor.matmul(peh, xt[:, k * 128:(k + 1) * 128],
                                         at[k][:, nsl],
                                         start=False, stop=(k == 7))
                    nm = stats.tile([128, 1], f32, tag="nmh")
                    nc.vector.tensor_reduce(
                        nm, peh, axis=mybir.AxisListType.X,
                        op=mybir.AluOpType.max, negate=True)
                    pehalves.append(peh)
                    nmh.append(nm)
                negmax = stats.tile([128, 1], f32)
                nc.vector.tensor_tensor(negmax, nmh[0], nmh[1],
                                        op=mybir.AluOpType.min)

                expv = epool.tile([128, N], f16)
                shalf = []
                for half in range(2):
                    nsl = slice(half * 512, (half + 1) * 512)
                    sh = stats.tile([128, 1], f32, tag="sh")
                    nc.scalar.activation(expv[:, nsl], pehalves[half], Exp,
                                         bias=negmax, scale=1.0, accum_out=sh)
                    shalf.append(sh)
                sums = stats.tile([128, 1], f32)
                nc.vector.tensor_add(sums, shalf[0], shalf[1])
                invs = stats.tile([128, 1], f32)
                nc.vector.reciprocal(invs, sums)
                diag = stats.tile([128, 128], f16, tag="diag")
                nc.vector.tensor_scalar_mul(diag, ident, invs)

                po = psO.tile([128, N], f32)
                for j in range(8):
                    jsl = slice(j * 128, (j + 1) * 128)
                    nc.tensor.matmul(po[:, jsl], expv[:, jsl], diag,
                                     start=True, stop=True)
                nc.scalar.copy(
                    out_sb[:, :, lb, :],
                    po.rearrange("p (j l) -> p j l", j=8))

                if b == BPC - 1:
                    # split the final batch's store so the tail drains early
                    nc.sync.dma_start(out=out_r[b, :, :, lb, :],
                                      in_=out_sb[:, :, lb, :])
            if b != BPC - 1:
                nc.sync.dma_start(out=out_r[b], in_=out_sb)


def _fold(inputs):
    FE = np.asarray(inputs["feature_embeddings"], np.float32)
    W_lin = np.asarray(inputs["W_lin"], np.float32)
    b_lin = np.asarray(inputs["b_lin"], np.float32)
    W_feat = np.asarray(inputs["W_feat"], np.float32)
    b_feat = np.asarray(inputs["b_feat"], np.float32)
    v = np.asarray(inputs["v"], np.float32)
    a1t = W_feat.T @ FE.T              # [2H, NF]
    a2t = W_lin.T @ v.T                # [2H, FP]
    at = np.concatenate([a1t, a2t], axis=1)          # [2H, N]
    c = np.concatenate([FE @ b_feat, v @ b_lin])     # [N]
    atw = np.ascontiguousarray(
        at.reshape(8, 128, N).astype(np.float16))
    wrow = c.astype(np.float16).reshape(1, N)  # bias row
    return atw, wrow


def _shards(inputs, host_fold=HOST_FOLD):
    x = np.ascontiguousarray(inputs["encoder_outputs"], dtype=np.float32)
    if host_fold:
        atw, wrow = _fold(inputs)
        rep = {"atw": atw, "wrow": wrow}
    else:
        rep = {
            "fe": np.ascontiguousarray(inputs["feature_embeddings"], np.float32),
            "wlin": np.ascontiguousarray(inputs["W_lin"], np.float32),
            "blin": np.ascontiguousarray(inputs["b_lin"], np.float32),
            "wfeat": np.ascontiguousarray(inputs["W_feat"], np.float32),
            "bfeat": np.ascontiguousarray(inputs["b_feat"], np.float32),
            "v": np.ascontiguousarray(inputs["v"], np.float32),
        }
    in_maps = []
    for c in range(NCORES):
        m = dict(rep)
        m["x"] = np.ascontiguousarray(x[:, c * BPC:(c + 1) * BPC, :])
        in_maps.append(m)
    return in_maps


def kernel(**inputs):
    from concourse.bass_utils import run_bass_kernel_spmd

    if "nc" not in _cache:
        _cache["nc"] = _build()
    nc = _cache["nc"]
    res = run_bass_kernel_spmd(nc, _shards(inputs), core_ids=list(range(NCORES)))
    out = np.concatenate([r["out"] for r in res.results], axis=0)
    return out.astype(np.float32)
